# revision 6
# baseline (speedup 1.0000x reference)
"""Llama attention (B=2, S=2048, H=2048, NH=32, NKV=8, D=64) on 8 trn2 cores.

Sharding: tensor-parallel over heads. Core c owns q-heads [4c, 4c+4) and
kv-head c (GQA groups stay aligned). Each core computes its partial
out_c = attn_c @ wo[:, 256c:256c+256].T over the full batch/sequence;
the host sums the 8 partials.

Device layout notes:
  - projections computed "feature-major": qT/kT [d, tok] via stationary
    weight tiles streaming xT; v is PE-transposed to token-major and
    augmented with a ones column so the AV matmul also produces softmax
    denominators.
  - scoresT[k, q] = kT.T @ qT (K=64), exp on ACT with fused *D^-0.5,
    causal handled by partial-width matmuls + one triangular mask mul
    on the diagonal 128x128 block.
  - softmax normalization: reciprocal of the sums row, broadcast across
    partitions with a K=1 float32r matmul, folded into the psum->sbuf copy.
"""

import sys

if "/opt/trn_rl_repo" not in sys.path:
    sys.path.insert(0, "/opt/trn_rl_repo")

import numpy as np
import ml_dtypes

import concourse.bass as bass
import concourse.mybir as mybir
import concourse.tile as tile
from concourse import bacc
from concourse.masks import make_identity

bf16 = mybir.dt.bfloat16
f16 = mybir.dt.float16
f32 = mybir.dt.float32
f32r = mybir.dt.float32r
AF = mybir.ActivationFunctionType

B = 2
D = 64
QH = 4                      # q heads per core
SCALE = D ** -0.5


def _segments(lo, hi, step=512):
    """Split [lo, hi) at multiples of `step` (matmul one-psum-bank limit)."""
    out = []
    while lo < hi:
        nxt = min(hi, (lo // step + 1) * step)
        out.append((lo, nxt))
        lo = nxt
    return out


def build(Sb=2048, H=2048, NGW=1024, QCW=1024):
    """Sb: tokens per batch; H: model dim; NGW: stage-1 token group width;
    QCW: stage-2 q-chunk width."""
    ST = B * Sb             # total tokens
    KT = H // 128           # contraction tiles for projections
    NKTB = Sb // 128        # k-token tiles per batch
    DQ = QH * D             # 256

    nc = bacc.Bacc(trn_type="TRN2")
    xT_d = nc.dram_tensor("xT", [H, ST], bf16, kind="ExternalInput")
    wqkv_d = nc.dram_tensor("wqkvT", [H, DQ + 2 * D], bf16, kind="ExternalInput")
    wo_d = nc.dram_tensor("woT", [DQ, H], bf16, kind="ExternalInput")
    cos2_d = nc.dram_tensor("cos2", [128, ST], bf16, kind="ExternalInput")
    sinadj_d = nc.dram_tensor("sinadj", [128, ST], bf16, kind="ExternalInput")
    out_d = nc.dram_tensor("out", [ST, H], f32, kind="ExternalOutput")

    with tile.TileContext(nc) as tc:
        with (
            tc.tile_pool(name="consts", bufs=1) as consts,
            tc.tile_pool(name="resident", bufs=1) as res,
            tc.tile_pool(name="xpool", bufs=3) as xpool,
            tc.tile_pool(name="scratch", bufs=3) as scratch,
            tc.tile_pool(name="etp", bufs=4) as etp,
            tc.tile_pool(name="npool", bufs=3) as npool,
            tc.tile_pool(name="obuf", bufs=3) as obuf,
        ):
            ident = consts.tile([D, D], bf16, name="ident")
            make_identity(nc, ident)
            ones64 = consts.tile([1, D], f16, name="ones64")
            nc.vector.memset(ones64[:], 1.0)
            trimask = consts.tile([128, 128], bf16, name="trimask")
            nc.vector.memset(trimask[:], 1.0)
            nc.gpsimd.affine_select(
                out=trimask[:], in_=trimask[:],
                compare_op=mybir.AluOpType.is_ge, fill=0.0,
                base=0, pattern=[[1, 128]], channel_multiplier=-1,
            )

            cos2 = res.tile([128, ST], bf16, name="cos2")
            nc.sync.dma_start(cos2[:], cos2_d[:])
            sinadj = res.tile([128, ST], bf16, name="sinadj")
            nc.sync.dma_start(sinadj[:], sinadj_d[:])

            wo_t = []
            for ki in range(DQ // 128):
                w = res.tile([128, H], bf16, name=f"wo{ki}")
                nc.sync.dma_start(w[:], wo_d[ki * 128:(ki + 1) * 128, :])
                wo_t.append(w)

            wqkv_t = []
            for kt in range(KT):
                w = res.tile([128, DQ + 2 * D], bf16, name=f"wqkv{kt}")
                nc.sync.dma_start(w[:], wqkv_d[kt * 128:(kt + 1) * 128, :])
                wqkv_t.append(w)

            qTh = [res.tile([D, ST], bf16, name=f"qT{h}") for h in range(QH)]
            kT = res.tile([D, ST], bf16, name="kT")
            attnT = [res.tile([128, ST], bf16, name=f"attnT{i}") for i in range(2)]
            vaug = [res.tile([128, D + 2], bf16, name=f"vaug{i}")
                    for i in range(ST // 128)]

            NM = DQ // 128 + 1  # q m-tiles + 1 kv tile

            # ---------------- stage 1: QKV projection + RoPE + v transpose
            with (
                tc.tile_pool(name="proj_ps", bufs=NM, space="PSUM") as proj_ps,
                tc.tile_pool(name="vt_ps", bufs=2, space="PSUM") as vt_ps,
            ):
                for ng in range(ST // NGW):
                    c0 = ng * NGW
                    ptiles = [proj_ps.tile([128, NGW], f32, name=f"pp{ng}_{m}",
                                           tag="pp") for m in range(NM)]
                    for kt in range(KT):
                        xt = xpool.tile([128, NGW], bf16, name=f"x{ng}_{kt}",
                                        tag="xt")
                        nc.sync.dma_start(xt[:], xT_d[kt * 128:(kt + 1) * 128,
                                                      c0:c0 + NGW])
                        for m in range(NM):
                            for lo, hi in _segments(0, NGW):
                                nc.tensor.matmul(
                                    ptiles[m][:, lo:hi],
                                    wqkv_t[kt][:, m * 128:(m + 1) * 128],
                                    xt[:, lo:hi],
                                    start=(kt == 0), stop=(kt == KT - 1),
                                )

                    def rope(dst_hi, dst_lo, src_sb, psrc, rows, c0=c0):
                        """RoPE rows [0, rows) of src_sb (bf16 copy of psum);
                        write result to dst slices."""
                        sh = scratch.tile([128, NGW], bf16, name=f"sh{ng}{rows}",
                                          tag="sh")
                        for r0 in range(0, rows, 64):
                            nc.vector.tensor_copy(sh[r0:r0 + 32, :],
                                                  src_sb[r0 + 32:r0 + 64, :])
                            nc.vector.tensor_copy(sh[r0 + 32:r0 + 64, :],
                                                  src_sb[r0:r0 + 32, :])
                        t1 = scratch.tile([128, NGW], bf16, name=f"t1{ng}{rows}",
                                          tag="t1")
                        nc.vector.tensor_mul(t1[0:rows, :], src_sb[0:rows, :],
                                             cos2[0:rows, c0:c0 + NGW])
                        t2 = scratch.tile([128, NGW], bf16, name=f"t2{ng}{rows}",
                                          tag="t2")
                        nc.vector.tensor_mul(t2[0:rows, :], sh[0:rows, :],
                                             sinadj[0:rows, c0:c0 + NGW])
                        for i, (dst, dr0) in enumerate([(dst_lo, 0),
                                                        (dst_hi, 64)][:rows // 64]):
                            nc.vector.tensor_add(dst, t1[dr0:dr0 + 64, :],
                                                 t2[dr0:dr0 + 64, :])

                    for m in range(DQ // 128):
                        qb = scratch.tile([128, NGW], bf16, name=f"qb{ng}{m}",
                                          tag="qb")
                        nc.scalar.copy(qb[:], ptiles[m][:])
                        rope(qTh[2 * m + 1][:, c0:c0 + NGW],
                             qTh[2 * m][:, c0:c0 + NGW], qb, ptiles[m], 128)

                    kvb = scratch.tile([128, NGW], bf16, name=f"kvb{ng}", tag="qb")
                    nc.scalar.copy(kvb[0:64, :], ptiles[NM - 1][0:64, :])
                    rope(None, kT[:, c0:c0 + NGW], kvb, ptiles[NM - 1], 64)
                    # v rows [64:128) of psum -> base-0 tile -> token-major vaug
                    vb = scratch.tile([64, NGW], bf16, name=f"vb{ng}", tag="vb")
                    nc.vector.tensor_copy(vb[:], ptiles[NM - 1][64:128, :])
                    for j in range(NGW // 128):
                        tb = ng * (NGW // 128) + j
                        vt = vt_ps.tile([128, D], bf16, name=f"vt{tb}", tag="vt")
                        nc.tensor.transpose(
                            vt[:], vb[:, j * 128:(j + 1) * 128], ident[:])
                        nc.vector.tensor_copy(vaug[tb][:, 0:D], vt[:])
                        nc.vector.memset(vaug[tb][:, D:D + 1], 1.0)

            # ---------------- stage 2: attention per (batch, head)
            with (
                tc.tile_pool(name="sc_ps", bufs=2, space="PSUM") as sc_ps,
                tc.tile_pool(name="acc_ps", bufs=2, space="PSUM") as acc_ps,
            ):
                for b in range(B):
                    b0 = b * Sb
                    for h in range(QH):
                        for qc in range(Sb // QCW):
                            q0 = b0 + qc * QCW
                            acc = acc_ps.tile([D + 1, QCW], f32,
                                              name=f"acc{b}{h}{qc}", tag="acc")
                            nkt = (qc + 1) * (QCW // 128)
                            for kt in range(nkt):
                                r = kt * 128 - qc * QCW
                                w0 = max(0, r)
                                sc = sc_ps.tile([128, QCW], f32,
                                                name=f"sc{b}{h}{qc}{kt}", tag="sc")
                                for lo, hi in _segments(w0, QCW):
                                    nc.tensor.matmul(
                                        sc[:, lo:hi],
                                        kT[:, b0 + kt * 128:b0 + (kt + 1) * 128],
                                        qTh[h][:, q0 + lo:q0 + hi],
                                        start=True, stop=True)
                                et = etp.tile([128, QCW], bf16,
                                              name=f"et{b}{h}{qc}{kt}", tag="et")
                                nc.scalar.activation(et[:, w0:QCW], sc[:, w0:QCW],
                                                     AF.Exp, scale=SCALE)
                                if r >= 0:
                                    nc.vector.tensor_mul(et[:, r:r + 128],
                                                         et[:, r:r + 128],
                                                         trimask[:])
                                for lo, hi in _segments(w0, QCW):
                                    nc.tensor.matmul(
                                        acc[:, lo:hi],
                                        vaug[(b0 // 128) + kt][:, 0:D + 1],
                                        et[:, lo:hi],
                                        start=(kt == 0), stop=(kt == nkt - 1))
                            recip = npool.tile([1, QCW], f32,
                                               name=f"rc{b}{h}{qc}", tag="recip")
                            nc.vector.reciprocal(recip[:], acc[D:D + 1, :])
                            reciph = npool.tile([1, QCW], f16,
                                                name=f"rh{b}{h}{qc}", tag="reciph")
                            nc.vector.tensor_copy(reciph[:], recip[:])
                            rb = sc_ps.tile([D, QCW], f32, name=f"rb{b}{h}{qc}",
                                            tag="sc")
                            for lo, hi in _segments(0, QCW):
                                nc.tensor.matmul(rb[:, lo:hi],
                                                 ones64[:],
                                                 reciph[:, lo:hi],
                                                 start=True, stop=True)
                            rbs = npool.tile([D, QCW], f32, name=f"rbs{b}{h}{qc}",
                                             tag="rbs")
                            nc.scalar.copy(rbs[:], rb[:])
                            hr = (h % 2) * 64
                            nc.vector.tensor_mul(
                                attnT[h // 2][hr:hr + 64, q0:q0 + QCW],
                                acc[0:D, :], rbs[:])

            # ---------------- stage 3: output projection
            with tc.tile_pool(name="o_ps", bufs=6, space="PSUM") as o_ps:
                hsegs = _segments(0, H)
                for tt in range(ST // 128):
                    t0 = tt * 128
                    otiles = [o_ps.tile([128, hi - lo], f32, name=f"ot{tt}_{hc}",
                                        tag="ot") for hc, (lo, hi) in enumerate(hsegs)]
                    for ki in range(DQ // 128):
                        for hc, (lo, hi) in enumerate(hsegs):
                            nc.tensor.matmul(
                                otiles[hc],
                                attnT[ki][:, t0:t0 + 128],
                                wo_t[ki][:, lo:hi],
                                start=(ki == 0), stop=(ki == DQ // 128 - 1))
                    ob = obuf.tile([128, H], f32, name=f"ob{tt}", tag="ob")
                    for hc, (lo, hi) in enumerate(hsegs):
                        nc.vector.tensor_copy(ob[:, lo:hi], otiles[hc])
                    nc.sync.dma_start(out_d[t0:t0 + 128, :], ob[:])

    nc.finalize()
    return nc


_CACHE = {}


def _get_nc(key, **kw):
    if key not in _CACHE:
        _CACHE[key] = build(**kw)
    return _CACHE[key]


def _prep_inputs(x, cos, sin, wq, wk, wv, wo):
    """Host-side sharding/layout prep. Returns list of 8 per-core in_maps."""
    Bx, S, H = x.shape
    bf = ml_dtypes.bfloat16
    x2d = x.reshape(Bx * S, H)
    xT = np.ascontiguousarray(x2d.T).astype(bf)

    cosT = np.concatenate([cos[b].T for b in range(Bx)], axis=1)   # [64, B*S]
    sinT = np.concatenate([sin[b].T for b in range(Bx)], axis=1)
    cos2 = np.tile(cosT, (2, 1)).astype(bf)
    sadj64 = np.concatenate([-sinT[0:32], sinT[32:64]], axis=0)
    sinadj = np.tile(sadj64, (2, 1)).astype(bf)

    in_maps = []
    for c in range(8):
        wq_c = wq[c * 256:(c + 1) * 256]          # (256, H)
        wk_c = wk[c * 64:(c + 1) * 64]            # (64, H)
        wv_c = wv[c * 64:(c + 1) * 64]
        wqkvT = np.concatenate([wq_c.T, wk_c.T, wv_c.T], axis=1).astype(bf)
        woT = np.ascontiguousarray(wo[:, c * 256:(c + 1) * 256].T).astype(bf)
        in_maps.append({
            "xT": xT, "cos2": cos2, "sinadj": sinadj,
            "wqkvT": np.ascontiguousarray(wqkvT),
            "woT": woT,
        })
    return in_maps


LAST_RESULTS = None


def kernel(x, cos, sin, mask, wq, wk, wv, wo):
    global LAST_RESULTS
    from concourse.bass_utils import run_bass_kernel_spmd

    x = np.asarray(x, dtype=np.float32)
    cos = np.asarray(cos, dtype=np.float32)
    sin = np.asarray(sin, dtype=np.float32)
    wq = np.asarray(wq, dtype=np.float32)
    wk = np.asarray(wk, dtype=np.float32)
    wv = np.asarray(wv, dtype=np.float32)
    wo = np.asarray(wo, dtype=np.float32)

    nc = _get_nc("full")
    in_maps = _prep_inputs(x, cos, sin, wq, wk, wv, wo)
    LAST_RESULTS = run_bass_kernel_spmd(nc, in_maps, core_ids=list(range(8)))
    Bx, S, H = x.shape
    out = np.zeros((Bx * S, H), dtype=np.float32)
    for r in LAST_RESULTS.results:
        out += r["out"]
    return out.reshape(Bx, S, H)


# revision 13
# speedup vs baseline: 1.2376x; 1.2376x over previous
"""Llama attention (B=2, S=2048, H=2048, NH=32, NKV=8, D=64) on 8 trn2 cores.

Sharding: tensor-parallel over heads. Core c owns q-heads [4c, 4c+4) and
kv-head c (GQA groups stay aligned). Each core computes its partial
out_c = attn_c @ wo[:, 256c:256c+256].T over the full batch/sequence in
bf16; the host sums the 8 partials in f32.

Device layout notes:
  - projections computed "feature-major": q/k [d, tok] via stationary
    weight tiles streaming xT; v is PE-transposed to token-major and
    augmented with a ones column so the AV matmul also produces softmax
    denominators.
  - q heads are stored pair-stacked ([head 2p | head 2p+1] on partitions)
    and k is duplicated into both partition halves, so the two scoresT
    matmuls of a pair run concurrently on disjoint PE row-groups (K=64
    each). One 3D-AP exp covers both heads.
  - causal: partial-width matmuls + a triangular mask multiply on the
    diagonal 128x128 block of each head.
  - softmax normalization: reciprocal_approx_fast on the sums row,
    broadcast across partitions with a K=1 fp16 matmul, folded into the
    psum->sbuf copy. The output projection is interleaved per q-chunk to
    keep the PE warm while ACT computes exponentials.
"""

import sys

if "/opt/trn_rl_repo" not in sys.path:
    sys.path.insert(0, "/opt/trn_rl_repo")

import numpy as np
import ml_dtypes

import concourse.bass as bass
import concourse.mybir as mybir
import concourse.tile as tile
from concourse import bacc
from concourse.masks import make_identity

bf16 = mybir.dt.bfloat16
f16 = mybir.dt.float16
f32 = mybir.dt.float32
AF = mybir.ActivationFunctionType

B = 2
D = 64
QH = 4                      # q heads per core
SCALE = D ** -0.5


def _segments(lo, hi, step=512):
    """Split [lo, hi) at multiples of `step` (matmul one-psum-bank limit)."""
    out = []
    while lo < hi:
        nxt = min(hi, (lo // step + 1) * step)
        out.append((lo, nxt))
        lo = nxt
    return out


def build(Sb=2048, H=2048, NGW=1024, QCW=512):
    """Sb: tokens per batch; H: model dim; NGW: stage-1 token group width;
    QCW: per-head q-chunk width in stage 2 (<= 512)."""
    assert QCW <= 512
    ST = B * Sb             # total tokens
    KT = H // 128           # contraction tiles for projections
    DQ = QH * D             # 256
    NP = QH // 2            # head pairs per core

    nc = bacc.Bacc(trn_type="TRN2")
    xT_d = nc.dram_tensor("xT", [H, ST], bf16, kind="ExternalInput")
    wqkv_d = nc.dram_tensor("wqkvT", [H, DQ + 2 * D], bf16, kind="ExternalInput")
    wo_d = nc.dram_tensor("woT", [DQ, H], bf16, kind="ExternalInput")
    cos2_d = nc.dram_tensor("cos2", [128, ST], bf16, kind="ExternalInput")
    sinadj_d = nc.dram_tensor("sinadj", [128, ST], bf16, kind="ExternalInput")
    out_d = nc.dram_tensor("out", [ST, H], bf16, kind="ExternalOutput")

    with tile.TileContext(nc) as tc:
        with (
            tc.tile_pool(name="consts", bufs=1) as consts,
            tc.tile_pool(name="resident", bufs=1) as res,
            tc.tile_pool(name="xpool", bufs=3) as xpool,
            tc.tile_pool(name="scratch", bufs=3) as scratch,
            tc.tile_pool(name="etp", bufs=6) as etp,
            tc.tile_pool(name="npool", bufs=3) as npool,
            tc.tile_pool(name="obuf", bufs=3) as obuf,
        ):
            ident = consts.tile([D, D], bf16, name="ident")
            make_identity(nc, ident)
            ones64 = consts.tile([1, D], f16, name="ones64")
            nc.vector.memset(ones64[:], 1.0)
            trimask = consts.tile([128, 128], bf16, name="trimask")
            nc.vector.memset(trimask[:], 1.0)
            nc.gpsimd.affine_select(
                out=trimask[:], in_=trimask[:],
                compare_op=mybir.AluOpType.is_ge, fill=0.0,
                base=0, pattern=[[1, 128]], channel_multiplier=-1,
            )

            cos2 = res.tile([128, ST], bf16, name="cos2")
            nc.sync.dma_start(cos2[:], cos2_d[:])
            sinadj = res.tile([128, ST], bf16, name="sinadj")
            nc.sync.dma_start(sinadj[:], sinadj_d[:])

            wo_t = []
            for ki in range(DQ // 128):
                w = res.tile([128, H], bf16, name=f"wo{ki}")
                nc.sync.dma_start(w[:], wo_d[ki * 128:(ki + 1) * 128, :])
                wo_t.append(w)

            wqkv_t = []
            for kt in range(KT):
                w = res.tile([128, DQ + 2 * D], bf16, name=f"wqkv{kt}")
                nc.sync.dma_start(w[:], wqkv_d[kt * 128:(kt + 1) * 128, :])
                wqkv_t.append(w)

            # pair-stacked q, duplicated k, pair-stacked attention output
            q2 = [res.tile([128, ST], bf16, name=f"q2_{p}") for p in range(NP)]
            k2 = res.tile([128, ST], bf16, name="k2")
            attnT = [res.tile([128, ST], bf16, name=f"attnT{p}") for p in range(NP)]
            vaug = [res.tile([128, D + 2], bf16, name=f"vaug{i}")
                    for i in range(ST // 128)]

            NM = NP + 1  # q pair m-tiles + 1 kv tile

            # ---------------- stage 1: QKV projection + RoPE + v transpose
            with (
                tc.tile_pool(name="proj_ps", bufs=NM, space="PSUM") as proj_ps,
                tc.tile_pool(name="vt_ps", bufs=2, space="PSUM") as vt_ps,
            ):
                for ng in range(ST // NGW):
                    c0 = ng * NGW
                    ptiles = [proj_ps.tile([128, NGW], f32, name=f"pp{ng}_{m}",
                                           tag="pp") for m in range(NM)]
                    for kt in range(KT):
                        xt = xpool.tile([128, NGW], bf16, name=f"x{ng}_{kt}",
                                        tag="xt")
                        nc.sync.dma_start(xt[:], xT_d[kt * 128:(kt + 1) * 128,
                                                      c0:c0 + NGW])
                        for m in range(NM):
                            for lo, hi in _segments(0, NGW):
                                nc.tensor.matmul(
                                    ptiles[m][:, lo:hi],
                                    wqkv_t[kt][:, m * 128:(m + 1) * 128],
                                    xt[:, lo:hi],
                                    start=(kt == 0), stop=(kt == KT - 1),
                                )

                    def rope_mats(src_sb, rows, tag, c0=c0, ng=ng):
                        """RoPE product terms for rows [0, rows) of src_sb."""
                        sh = scratch.tile([128, NGW], bf16, name=f"sh{ng}{tag}",
                                          tag="sh")
                        for r0 in range(0, rows, 64):
                            nc.vector.tensor_copy(sh[r0:r0 + 32, :],
                                                  src_sb[r0 + 32:r0 + 64, :])
                            nc.vector.tensor_copy(sh[r0 + 32:r0 + 64, :],
                                                  src_sb[r0:r0 + 32, :])
                        t1 = scratch.tile([128, NGW], bf16, name=f"t1{ng}{tag}",
                                          tag="t1")
                        nc.vector.tensor_mul(t1[0:rows, :], src_sb[0:rows, :],
                                             cos2[0:rows, c0:c0 + NGW])
                        t2 = scratch.tile([128, NGW], bf16, name=f"t2{ng}{tag}",
                                          tag="t2")
                        nc.vector.tensor_mul(t2[0:rows, :], sh[0:rows, :],
                                             sinadj[0:rows, c0:c0 + NGW])
                        return t1, t2

                    for m in range(NP):
                        qb = scratch.tile([128, NGW], bf16, name=f"qb{ng}{m}",
                                          tag="qb")
                        nc.scalar.copy(qb[:], ptiles[m][:])
                        t1, t2 = rope_mats(qb, 128, f"q{m}")
                        nc.vector.tensor_add(q2[m][:, c0:c0 + NGW], t1[:], t2[:])

                    kvb = scratch.tile([128, NGW], bf16, name=f"kvb{ng}", tag="qb")
                    nc.scalar.copy(kvb[0:64, :], ptiles[NM - 1][0:64, :])
                    t1, t2 = rope_mats(kvb, 64, "k")
                    nc.vector.tensor_add(k2[0:64, c0:c0 + NGW],
                                         t1[0:64, :], t2[0:64, :])
                    nc.vector.tensor_add(k2[64:128, c0:c0 + NGW],
                                         t1[0:64, :], t2[0:64, :])
                    # v rows [64:128) of psum -> base-0 tile -> token-major vaug
                    vb = scratch.tile([64, NGW], bf16, name=f"vb{ng}", tag="vb")
                    nc.vector.tensor_copy(vb[:], ptiles[NM - 1][64:128, :])
                    for j in range(NGW // 128):
                        tb = ng * (NGW // 128) + j
                        vt = vt_ps.tile([128, D], bf16, name=f"vt{tb}", tag="vt")
                        nc.tensor.transpose(
                            vt[:], vb[:, j * 128:(j + 1) * 128], ident[:])
                        nc.vector.tensor_copy(vaug[tb][:, 0:D], vt[:])
                        nc.vector.memset(vaug[tb][:, D:D + 1], 1.0)

            # ---------------- stage 2+3: attention (pair-packed) + out proj
            NQC = Sb // QCW
            NTT = QCW // 128        # token tiles per q-chunk
            hsegs = _segments(0, H)
            with (
                tc.tile_pool(name="sc_ps", bufs=2, space="PSUM") as sc_ps,
                tc.tile_pool(name="acc_ps", bufs=1, space="PSUM") as acc_ps,
                tc.tile_pool(name="o_ps", bufs=2, space="PSUM") as o_ps,
            ):
                for b in range(B):
                    b0 = b * Sb
                    for qc in range(NQC):
                        q0 = b0 + qc * QCW
                        nkt = (qc + 1) * (QCW // 128)
                        for p in range(NP):
                            accs = [acc_ps.tile([D + 1, QCW], f32,
                                                name=f"acc{b}{p}{qc}{h}",
                                                tag=f"acc{h}")
                                    for h in range(2)]
                            for kt in range(nkt):
                                r = kt * 128 - qc * QCW
                                w0 = max(0, r)
                                sc = sc_ps.tile([128, 2 * QCW], f32,
                                                name=f"sc{b}{p}{qc}{kt}",
                                                tag="sc")
                                kcols = slice(b0 + kt * 128, b0 + (kt + 1) * 128)
                                for h in range(2):
                                    hr = h * 64
                                    nc.tensor.matmul(
                                        sc[:, h * QCW + w0:(h + 1) * QCW],
                                        k2[hr:hr + 64, kcols],
                                        q2[p][hr:hr + 64, q0 + w0:q0 + QCW],
                                        start=True, stop=True)
                                et = etp.tile([128, 2 * QCW], bf16,
                                              name=f"et{b}{p}{qc}{kt}", tag="et")
                                sc3 = sc[:].rearrange("p (h w) -> p h w", h=2)
                                et3 = et[:].rearrange("p (h w) -> p h w", h=2)
                                nc.scalar.activation(et3[:, :, w0:QCW],
                                                     sc3[:, :, w0:QCW],
                                                     AF.Exp, scale=SCALE)
                                if r >= 0:
                                    for h in range(2):
                                        o = h * QCW + r
                                        nc.vector.tensor_mul(
                                            et[:, o:o + 128],
                                            et[:, o:o + 128], trimask[:])
                                va = vaug[(b0 // 128) + kt]
                                for h in range(2):
                                    nc.tensor.matmul(
                                        accs[h][:, w0:QCW],
                                        va[:, 0:D + 1],
                                        et[:, h * QCW + w0:(h + 1) * QCW],
                                        start=(kt == 0), stop=(kt == nkt - 1),
                                        skip_group_check=True)
                            for h in range(2):
                                acc = accs[h]
                                recip = npool.tile([1, QCW], f32,
                                                   name=f"rc{b}{p}{qc}{h}",
                                                   tag="recip")
                                nc.vector.reciprocal(recip[:],
                                                     acc[D:D + 1, :])
                                reciph = npool.tile([1, QCW], f16,
                                                    name=f"rh{b}{p}{qc}{h}",
                                                    tag="reciph")
                                nc.vector.tensor_copy(reciph[:], recip[:])
                                rb = sc_ps.tile([D, QCW], f32,
                                                name=f"rb{b}{p}{qc}{h}", tag="sc")
                                for lo, hi in _segments(0, QCW):
                                    nc.tensor.matmul(rb[:, lo:hi], ones64[:],
                                                     reciph[:, lo:hi],
                                                     start=True, stop=True)
                                rbs = npool.tile([D, QCW], f32,
                                                 name=f"rbs{b}{p}{qc}{h}",
                                                 tag="rbs")
                                nc.scalar.copy(rbs[:], rb[:])
                                nc.vector.tensor_mul(
                                    attnT[p][h * 64:h * 64 + 64, q0:q0 + QCW],
                                    acc[0:D, :], rbs[:])
                        # ---- out projection for this q-chunk's tokens
                        for tl in range(NTT):
                            t0 = q0 + tl * 128
                            ob = obuf.tile([128, H], bf16, name=f"ob{b}{qc}{tl}",
                                           tag="ob")
                            for hc, (lo, hi) in enumerate(hsegs):
                                ot = o_ps.tile([128, hi - lo], f32,
                                               name=f"ot{b}{qc}{tl}{hc}",
                                               tag="ot")
                                for ki in range(NP):
                                    nc.tensor.matmul(
                                        ot[:],
                                        attnT[ki][:, t0:t0 + 128],
                                        wo_t[ki][:, lo:hi],
                                        start=(ki == 0), stop=(ki == NP - 1))
                                nc.vector.tensor_copy(ob[:, lo:hi], ot[:])
                            nc.sync.dma_start(out_d[t0:t0 + 128, :], ob[:])

    nc.finalize()
    return nc


_CACHE = {}


def _get_nc(key, **kw):
    if key not in _CACHE:
        _CACHE[key] = build(**kw)
    return _CACHE[key]


def _prep_inputs(x, cos, sin, wq, wk, wv, wo):
    """Host-side sharding/layout prep. Returns list of 8 per-core in_maps."""
    Bx, S, H = x.shape
    bf = ml_dtypes.bfloat16
    x2d = x.reshape(Bx * S, H)
    xT = np.ascontiguousarray(x2d.T).astype(bf)

    cosT = np.concatenate([cos[b].T for b in range(Bx)], axis=1)   # [64, B*S]
    sinT = np.concatenate([sin[b].T for b in range(Bx)], axis=1)
    cos2 = np.tile(cosT, (2, 1)).astype(bf)
    sadj64 = np.concatenate([-sinT[0:32], sinT[32:64]], axis=0)
    sinadj = np.tile(sadj64, (2, 1)).astype(bf)

    in_maps = []
    for c in range(8):
        wq_c = wq[c * 256:(c + 1) * 256]          # (256, H)
        wk_c = wk[c * 64:(c + 1) * 64]            # (64, H)
        wv_c = wv[c * 64:(c + 1) * 64]
        wqkvT = np.concatenate([wq_c.T, wk_c.T, wv_c.T], axis=1).astype(bf)
        woT = np.ascontiguousarray(wo[:, c * 256:(c + 1) * 256].T).astype(bf)
        in_maps.append({
            "xT": xT, "cos2": cos2, "sinadj": sinadj,
            "wqkvT": np.ascontiguousarray(wqkvT),
            "woT": woT,
        })
    return in_maps


LAST_RESULTS = None


def kernel(x, cos, sin, mask, wq, wk, wv, wo):
    global LAST_RESULTS
    from concourse.bass_utils import run_bass_kernel_spmd

    x = np.asarray(x, dtype=np.float32)
    cos = np.asarray(cos, dtype=np.float32)
    sin = np.asarray(sin, dtype=np.float32)
    wq = np.asarray(wq, dtype=np.float32)
    wk = np.asarray(wk, dtype=np.float32)
    wv = np.asarray(wv, dtype=np.float32)
    wo = np.asarray(wo, dtype=np.float32)

    nc = _get_nc("full")
    in_maps = _prep_inputs(x, cos, sin, wq, wk, wv, wo)
    LAST_RESULTS = run_bass_kernel_spmd(nc, in_maps, core_ids=list(range(8)))
    Bx, S, H = x.shape
    out = np.zeros((Bx * S, H), dtype=np.float32)
    for r in LAST_RESULTS.results:
        out += r["out"].astype(np.float32)
    return out.reshape(Bx, S, H)
